# revision 24
# baseline (speedup 1.0000x reference)
"""Trainium2 Bass kernel for nn_AttentionScorer (sparse_attention).

Math (per batch b):
    S = (v_hat @ t_hat.T) / TEMP                  [Lv, Lt]
    E = exp(S)            (softmax shift terms cancel exactly)
    R[v] = sum_t E[v,t]   row sums
    C[t] = sum_v E[v,t]   col sums
    TS[t] = sum_v E[v,t]/R[v]                     (text_score * Lv)
    w = TS >= mean(TS); cnt = sum(w)
    vs[v] = sum_t E[v,t] * w[t] / C[t] / cnt
    out = (vs - min) / (max - min + eps)

Device layout: data-parallel over B across 8 cores (2 batches/core).
On-chip value is E' = exp(S)-1 stored fp16 (|E'| ~ 0.02 keeps ~45x
more absolute precision than storing exp itself). All downstream
terms absorb the shift exactly.

v2 engine budget per batch (errata-calibrated):
  PE   : S matmuls (256x216ns) + CT sweep (64x216) + g broadcast  ~70us
  ACT  : exp in-place on PSUM (32x1.0us) + half the -1/fp16
         downcasts (Copy bias=-1 w/ accum_out -> R' row sums)     ~52us
  DVE  : other half of downcasts (tensor_scalar w/ accum_out),
         fused pass-2 tensor_tensor_reduce tiles, stage B         ~35us
  GPS  : pass-2 scalar_tensor_tensor tiles (SBUF-only engine)     ~34us
R' = R - LT comes free from the downcast accum_out, so exp carries
no accumulator read.  The threshold is mean_t(TS row) = a direct
[1,LT] reduce of the DMA'd TS row (identical data as the compare, so
no identity-consistency concerns).  The previous batch's stage-B +
pass-2 is emitted in per-quad slices inside the next batch's phase 1
(stage-B DVE part at quad 0, its PE matmuls at quad 1, pass-2 tiles
at quads 2..7) so no engine FIFO ever gets a multi-10us block that
would stall PSUM recycling, and the last batch's tail is split
DVE||GPSIMD to compress the exposed serial region.
"""

import numpy as np
from contextlib import ExitStack

import concourse.bacc as bacc
import concourse.mybir as mybir
import concourse.tile as tile
from concourse.bass_utils import run_bass_kernel_spmd

B, LT, LV, D = 16, 1024, 4096, 512
NCORES = 8
BC = B // NCORES  # batches per core
KT = D // 128     # contraction tiles
NT = LT // 512    # 512-wide output chunks
NV = LV // 128    # v row tiles
QV = NV // 4      # quad tiles (4 v-tiles each)
KSCALE = 32768.0  # keeps g = w*K/C in fp16 normal range
X0R = 1.0 / LT          # Newton seed for 1/R  (R = R' + LT, R'~ +-4)
X0C = 1.0 / (LV + 1.0)  # Newton seed for 1/C  (C = C' + LV)
F32 = mybir.dt.float32
FP16 = mybir.dt.float16

# tuning knobs
GP_TAIL_COVERED = 16   # pass-2 tiles on GPSIMD when tail overlaps next batch
GP_TAIL_FINAL = 7      # pass-2 tiles on GPSIMD in the exposed final tail
DVE_DOWNCAST = 16      # of 32 tiles per batch; rest on ACT

_CACHE = {}


def _build():
    nc = bacc.Bacc(
        "TRN2",
        target_bir_lowering=False,
        debug=False,
        enable_asserts=True,
        num_devices=NCORES,
    )
    vT = nc.dram_tensor("vT", [BC, D, LV], FP16, kind="ExternalInput").ap()
    tT = nc.dram_tensor("tT", [BC, D, LT], FP16, kind="ExternalInput").ap()
    vs_out = nc.dram_tensor("vs_out", [BC, 128, NV], F32, kind="ExternalOutput").ap()
    cnt_out = nc.dram_tensor("cnt_out", [BC, 1], F32, kind="ExternalOutput").ap()

    AF = mybir.ActivationFunctionType
    OP = mybir.AluOpType
    AX = mybir.AxisListType

    with tile.TileContext(nc) as tc, ExitStack() as ctx:
        tt_pool = ctx.enter_context(tc.tile_pool(name="tt", bufs=8))
        vt_pool = ctx.enter_context(tc.tile_pool(name="vt", bufs=16))
        e_pool = ctx.enter_context(tc.tile_pool(name="E", bufs=12))
        g_pool = ctx.enter_context(tc.tile_pool(name="g", bufs=2))
        row_pool = ctx.enter_context(tc.tile_pool(name="row", bufs=2))
        small = ctx.enter_context(tc.tile_pool(name="small", bufs=2))
        # PSUM: 3x [128,LT] rotation (6 banks) shared by S tiles + the g
        # broadcast, + one 2-bank slot for the CT sweep accumulator
        ps_pool = ctx.enter_context(tc.tile_pool(name="ps", bufs=3, space="PSUM"))
        ct_pool = ctx.enter_context(tc.tile_pool(name="ct", bufs=1, space="PSUM"))
        one_pool = ctx.enter_context(tc.tile_pool(name="one", bufs=1))

        ones_row = one_pool.tile([1, 128], FP16, tag="ones")
        nc.vector.memset(ones_row[:], 1.0)

        def make_tail(b, psum_ct, equads, vs_cols, is_final):
            """Return a list of 8 slice-emitters for batch b's stage-B +
            pass-2.  Covered tails get spliced one slice per quad into the
            next batch's phase 1; the final tail runs them back to back."""
            st = {}

            def slice_a():
                # TS sits on partition 0 of psum_ct (stationary is
                # [invR | ones]) so the whole threshold chain reads PSUM
                # directly.  The C row (partition 1) takes the slow path --
                # engines can't address base partition 1, and DMA can't read
                # PSUM, so: ACT copies [2,LT] to SBUF, DMA extracts row 1.
                # w is computed here (not slice_b) so psum_ct frees early
                # enough that the NEXT batch's CT quad-0 matmuls never stall.
                ct_sb = row_pool.tile([2, LT], F32, tag="ctsb")
                nc.scalar.activation(ct_sb[:], psum_ct[:], AF.Copy)
                c_row = row_pool.tile([1, LT], F32, tag="crow")
                nc.gpsimd.dma_start(c_row[:], ct_sb[1:2, :])
                # threshold = mean_t(TS row): direct reduce of the same
                # values the compare reads, so mask decisions track the
                # reference up to reduce-order rounding only
                th = small.tile([1, 1], F32, tag="th")
                nc.vector.tensor_reduce(th[:], psum_ct[0:1, :], AX.X, OP.add)
                nc.vector.tensor_scalar(th[:], th[:], 1.0 / LT, None, OP.mult)
                w_t = row_pool.tile([1, LT], F32, tag="w")
                nc.vector.tensor_scalar(
                    w_t[:], psum_ct[0:1, :], th[:], KSCALE, OP.is_ge, op1=OP.mult
                )
                st["c_row"] = c_row
                st["w_t"] = w_t

            def slice_b():
                c_row, w_t = st["c_row"], st["w_t"]
                # 1/C via one Newton step from a constant seed, as an ACT
                # affine (C = c_row + Lv concentrated around Lv)
                inv_c = row_pool.tile([1, LT], F32, tag="invc")
                nc.scalar.activation(
                    inv_c[:], c_row[:], AF.Copy,
                    scale=-X0C * X0C, bias=2.0 * X0C - LV * X0C * X0C,
                )
                g16 = row_pool.tile([1, LT], FP16, tag="g16")
                nc.vector.tensor_tensor(g16[:], w_t[:], inv_c[:], op=OP.mult)
                # broadcast g across 128 partitions via rank-1 matmul into a
                # ps_pool slot (ct_pool may already hold the NEXT batch's CT
                # accumulator at this point)
                ps_g = ps_pool.tile([128, LT], F32, tag="ps")
                for n in range(NT):
                    nc.tensor.matmul(
                        ps_g[:, n * 512:(n + 1) * 512],
                        lhsT=ones_row[:],
                        rhs=g16[:, n * 512:(n + 1) * 512],
                        start=True,
                        stop=True,
                    )
                g_rep = g_pool.tile([128, LT], FP16, tag="grep")
                nc.vector.tensor_copy(g_rep[:], ps_g[:])
                st["g_rep"] = g_rep
                cnt_t = small.tile([1, 1], F32, tag="cnt")
                nc.vector.tensor_reduce(cnt_t[:], w_t[:], AX.X, OP.add)
                nc.gpsimd.dma_start(cnt_out[b:b + 1, :], cnt_t[:])

            # pass-2 engine plan.  Fused DVE ops lose fp16 packing (STT
            # 1.74us/tile; tensor_tensor_reduce wedges the device), and
            # GPSIMD streaming halves concurrent DVE throughput (shared
            # SBUF port) -- so pass-2 is DVE tensor_tensor multiplies
            # (2.3us/quad) plus per-tile reduces split DVE (1.2us) / ACT
            # (Copy w/ accum_out, 1.4us).  The reduce has NO fp16 packing
            # on either engine, so the split ratio just balances load.
            # Covered tails return a flat op list that phase 1 injects ONE
            # op per tile slot: chunky per-quad injection queued ahead of
            # the PSUM-draining exp/downcast ops and stalled PE ~2-3us per
            # occurrence.
            if is_final:
                # ACT takes 22 tile-reduces, DVE 10, ACT quads' multiplies
                # emitted first so both engines stream in parallel
                act_tiles = set(range(12, NV))
                order = [2, 3, 4, 5, 6, 7, 0, 1]
            else:
                # covered tails put NO work on ACT: anything injected into
                # ACT's queue delays exp (the PSUM-critical op) and stalls
                # PE within ps_pool's ~2-tile slack.  DVE absorbs it all
                # and its small spill past phase end overlaps the endgame.
                act_tiles = set()
                order = list(range(QV))

            ops = []
            for q in order:
                def mult_q(q=q):
                    g_rep = st["g_rep"]
                    eq3 = equads[q][:].rearrange("p (q x) -> p q x", x=LT)
                    g_rep4 = g_rep[:].rearrange(
                        "p (y x) -> p y x", y=1
                    ).broadcast_to([128, 4, LT])
                    nc.vector.tensor_tensor(eq3, eq3, g_rep4, op=OP.mult)
                ops.append(mult_q)
                for i in range(4 * q, 4 * q + 4):
                    def red_i(q=q, i=i):
                        e_sl = equads[q][:, (i % 4) * LT:(i % 4 + 1) * LT]
                        if i in act_tiles:
                            nc.scalar.activation(
                                e_sl, e_sl, AF.Copy,
                                accum_out=vs_cols[:, i:i + 1],
                            )
                        else:
                            nc.vector.tensor_reduce(
                                vs_cols[:, i:i + 1], e_sl, AX.X, OP.add
                            )
                    ops.append(red_i)
            ops.append(lambda: nc.gpsimd.dma_start(vs_out[b, :, :], vs_cols[:]))

            return {"a": slice_a, "b": slice_b, "ops": ops}

        pending = None
        for b in range(BC):
            # interleave t-tiles with the first v chunk group so the first
            # matmul's two inputs are the first two DMA issues
            tts = []
            vchunks = {}
            for k in range(KT):
                tt_t = tt_pool.tile([128, LT], FP16, tag="tt")
                nc.sync.dma_start(tt_t[:], tT[b, k * 128:(k + 1) * 128, :])
                tts.append(tt_t)
                vt_t = vt_pool.tile([128, 1024], FP16, tag="vt")
                nc.sync.dma_start(vt_t[:], vT[b, k * 128:(k + 1) * 128, 0:1024])
                vchunks[(0, k)] = vt_t

            rmat = small.tile([128, NV], F32, tag="rmat")      # R' = R - LT
            inv_r = small.tile([128, NV], F32, tag="invr")
            lhs_all = small.tile([128, 2 * NV], FP16, tag="lhsall")
            nc.vector.memset(lhs_all[:], 1.0)
            vs_cols = small.tile([128, NV], F32, tag="vs")
            psum_ct = None
            equads = []
            n_dve_dc = 0
            pending_idx = 0

            def emit_ct(q, psum_ct, equads=equads, lhs_all=lhs_all):
                for ii in range(4 * q, 4 * q + 4):
                    ee = equads[q][:, (ii % 4) * LT:(ii % 4 + 1) * LT]
                    for n in range(NT):
                        nc.tensor.matmul(
                            psum_ct[:, n * 512:(n + 1) * 512],
                            lhsT=lhs_all[:, 2 * ii:2 * ii + 2],
                            rhs=ee[:, n * 512:(n + 1) * 512],
                            start=(ii == 0),
                            stop=(ii == NV - 1),
                            skip_group_check=True,
                        )

            for i in range(NV):
                c = i // 8
                if (c, 0) not in vchunks:
                    for k in range(KT):
                        vt_t = vt_pool.tile([128, 1024], FP16, tag="vt")
                        nc.sync.dma_start(
                            vt_t[:],
                            vT[b, k * 128:(k + 1) * 128, c * 1024:(c + 1) * 1024],
                        )
                        vchunks[(c, k)] = vt_t
                off = (i % 8) * 128
                ps = ps_pool.tile([128, LT], F32, tag="ps")
                for n in range(NT):
                    for k in range(KT):
                        nc.tensor.matmul(
                            ps[:, n * 512:(n + 1) * 512],
                            lhsT=vchunks[(c, k)][:, off:off + 128],
                            rhs=tts[k][:, n * 512:(n + 1) * 512],
                            start=(k == 0),
                            stop=(k == KT - 1),
                        )
                if i % 4 == 0:
                    e_q = e_pool.tile([128, 4 * LT], FP16, tag="E")
                    equads.append(e_q)
                # exp in-place on PSUM; the -1 downcast produces E' fp16 AND
                # R' row sums via accum_out, alternating ACT/DVE so neither
                # engine paces phase 1
                nc.scalar.activation(ps[:], ps[:], AF.Exp, scale=0.5)
                e_sl = e_q[:, (i % 4) * LT:(i % 4 + 1) * LT]
                if b == 0:
                    dve_dc = i % 2 == 0 and i not in (28, 30)
                else:
                    # tail-carrying batches keep only 8 downcasts on DVE --
                    # the covered tail already fills it; ACT (exp + copies)
                    # stays just under the PE pace
                    dve_dc = i % 4 == 0 and i <= 24 or i == 26
                if dve_dc:
                    # accum_out turns this into TensorScalarPtrReduce, which
                    # requires an explicit 2nd op.  Tiles 28-31 always go to
                    # ACT: at phase end DVE still holds tail backlog, while
                    # ACT drains promptly -- and the endgame CT sweep needs
                    # quad 7's E' as early as possible.
                    nc.vector.tensor_scalar(
                        e_sl, ps[:], -1.0, 0.0, OP.add, op1=OP.add,
                        accum_out=rmat[:, i:i + 1],
                    )
                else:
                    nc.scalar.activation(
                        e_sl, ps[:], AF.Copy, bias=-1.0,
                        accum_out=rmat[:, i:i + 1],
                    )

                if i % 4 == 3:
                    q = i // 4
                    # 1/R for THIS quad via one Newton step from the R'
                    # accum (inv = x0 - x0^2*R'), fp16 interleaved
                    # [invR | ones] stationary (TS lands on psum partition
                    # 0, C on partition 1).  Emitted BEFORE the tail splice:
                    # the deferred CT matmuls depend on lhs_all, and queueing
                    # this copy behind a tail slice once delayed quad 7's CT
                    # sweep by 13us
                    nc.vector.tensor_scalar(
                        inv_r[:, 4 * q:4 * q + 4], rmat[:, 4 * q:4 * q + 4],
                        -X0R * X0R, X0R, OP.mult, op1=OP.add,
                    )
                    nc.scalar.copy(
                        lhs_all[:, 8 * q:8 * q + 8]
                        .rearrange("p (i two) -> p i two", two=2)[:, :, 0:1],
                        inv_r[:, 4 * q:4 * q + 4]
                        .rearrange("p (i one) -> p i one", one=1),
                    )
                    # CT matmuls for the PREVIOUS quad: deferred one quad so
                    # the PE FIFO never sits behind this quad's last
                    # downcast (that dependency cost ~1-3us at every quad
                    # boundary when emitted in-quad)
                    if q > 0:
                        if psum_ct is None:
                            psum_ct = ct_pool.tile([2, LT], F32, tag="ctg")
                        emit_ct(q - 1, psum_ct)
                # splice the previous batch's tail: stage slices at tiles
                # 3/7, then the pass-2 op list paced ~2 ops per tile over
                # tiles 8..29 (tiles 30-31 stay clean so the endgame chain
                # never queues behind tail work)
                if pending is not None:
                    if i == 3:
                        pending["a"]()
                    elif i == 7:
                        pending["b"]()
                    elif i >= 8:
                        L = len(pending["ops"])
                        tgt = L if i >= 31 else (L * (i - 7)) // 24
                        while pending_idx < tgt:
                            pending["ops"][pending_idx]()
                            pending_idx += 1
                        if i >= 31:
                            pending = None

            psum_ct_f = psum_ct
            emit_ct(QV - 1, psum_ct_f)
            pending = make_tail(b, psum_ct_f, equads, vs_cols, b == BC - 1)

        pending["a"]()
        pending["b"]()
        for f in pending["ops"]:
            f()

    nc.compile()
    return nc


def _get_nc():
    if "nc" not in _CACHE:
        _CACHE["nc"] = _build()
    return _CACHE["nc"]


def _prep(t, v):
    t = np.nan_to_num(np.asarray(t, np.float32))
    v = np.nan_to_num(np.asarray(v, np.float32))
    t = t / np.maximum(np.linalg.norm(t, axis=-1, keepdims=True), 1e-6)
    v = v / np.maximum(np.linalg.norm(v, axis=-1, keepdims=True), 1e-6)
    tTh = np.ascontiguousarray(t.transpose(0, 2, 1)).astype(np.float16)
    vTh = np.ascontiguousarray(v.transpose(0, 2, 1)).astype(np.float16)
    in_maps = []
    for c in range(NCORES):
        sl = slice(c * BC, (c + 1) * BC)
        in_maps.append({"vT": vTh[sl], "tT": tTh[sl]})
    return in_maps


def _postprocess(res):
    out = np.empty((B, LV), np.float32)
    for c in range(NCORES):
        vs_dev = res.results[c]["vs_out"].astype(np.float64)  # [BC, 128, NV]
        kcnt = res.results[c]["cnt_out"].astype(np.float64)   # [BC, 1], = K * cnt
        vs = vs_dev.transpose(0, 2, 1).reshape(BC, LV)
        vs = vs / kcnt
        mn = vs.min(axis=1, keepdims=True)
        mx = vs.max(axis=1, keepdims=True)
        out[c * BC:(c + 1) * BC] = ((vs - mn) / (mx - mn + 1e-6)).astype(np.float32)
    return out


def _execute(t, v, **spmd_kwargs):
    nc = _get_nc()
    in_maps = _prep(t, v)
    res = run_bass_kernel_spmd(nc, in_maps, core_ids=list(range(NCORES)), **spmd_kwargs)
    return _postprocess(res), res


def kernel(t, v):
    out, _ = _execute(t, v)
    return out
